# revision 16
# baseline (speedup 1.0000x reference)
"""Trainium2 Bass kernel for 10-layer LSTM + additive attention pooling + FC.

Sharding: data-parallel over batch (8 cores x 32). Per core all 10 layers run
as a wavefront (layer l computes step t = tick - l); layers are banded
4-per-PSUM-bank into 3 groups. All matmul operands are fp16. The g-gate's
tanh is evaluated through the same sigmoid instruction as i/f/o (its weights
are pre-scaled by 2 on the host; tanh(z) = 2*sigmoid(2z) - 1 is fixed up as
2*(i*s) - i on the Pool engine), so each group needs only 2 Activation
instructions per tick. Bias is preloaded into PSUM by a K=4 one-hot matmul
before the x/h matmuls accumulate. h is transposed back to H-major on the PE
one tick behind the cell (software pipelined).
"""
import sys
import numpy as np

B, S, IN, H, OUT, L = 256, 512, 27, 128, 7, 10
NCORES = 8
BC = B // NCORES  # 32
G4 = 4 * H        # 512

for _p in ("/opt/trn_rl_repo",):
    if _p not in sys.path:
        sys.path.insert(0, _p)

_CACHE = {}


def _build(S_run):
    from contextlib import ExitStack
    import concourse.bass as bass
    import concourse.tile as tile
    from concourse import bacc, mybir
    from concourse.masks import make_identity

    f32 = mybir.dt.float32
    fp16 = mybir.dt.float16
    NT = S_run + L - 1

    nc = bacc.Bacc("TRN2", target_bir_lowering=False, debug=False,
                   enable_asserts=False, num_devices=NCORES)

    d_x = nc.dram_tensor("x", [IN + 1, S_run * BC], fp16, kind="ExternalInput").ap()
    d_w0 = nc.dram_tensor("w0", [IN + 1, G4], fp16, kind="ExternalInput").ap()
    d_wx = nc.dram_tensor("wx", [128, 9 * G4], fp16, kind="ExternalInput").ap()
    d_wh = nc.dram_tensor("wh", [128, 10 * G4], fp16, kind="ExternalInput").ap()
    d_bias4 = nc.dram_tensor("bias4", [4, 3 * G4], fp16,
                             kind="ExternalInput").ap()
    d_bsel = nc.dram_tensor("bsel", [4, 128], fp16, kind="ExternalInput").ap()
    d_attn = nc.dram_tensor("attn_wT", [128, 128], fp16, kind="ExternalInput").ap()
    d_attnb = nc.dram_tensor("attn_b", [128, 1], f32, kind="ExternalInput").ap()
    d_vw = nc.dram_tensor("v_w", [128, 1], fp16, kind="ExternalInput").ap()
    d_fcw = nc.dram_tensor("fc_wT", [128, OUT], f32, kind="ExternalInput").ap()
    d_fcb = nc.dram_tensor("fc_b", [1, OUT], f32, kind="ExternalInput").ap()
    d_out = nc.dram_tensor("out", [OUT, BC], f32, kind="ExternalOutput").ap()
    import os as _os
    dbg = _os.environ.get("DEBUG_HS9") == "1"
    d_hs9 = (nc.dram_tensor("hs9", [128, S_run * BC], fp16,
                            kind="ExternalOutput").ap() if dbg else None)

    Sig = mybir.ActivationFunctionType.Sigmoid
    Tanh = mybir.ActivationFunctionType.Tanh
    Exp = mybir.ActivationFunctionType.Exp
    MUL = mybir.AluOpType.mult
    ADD = mybir.AluOpType.add

    with tile.TileContext(nc) as tc:
        with ExitStack() as octx:
            keep = octx.enter_context(tc.tile_pool(name="keep", bufs=1))
            hs9 = keep.tile([128, S_run * BC], fp16)
            ident = keep.tile([128, 128], fp16)
            make_identity(nc, ident[:])

            # ================= recurrent phase =================
            with ExitStack() as ctx:
                stat = ctx.enter_context(tc.tile_pool(name="stat", bufs=1))
                xT = stat.tile([IN + 1, S_run * BC], fp16)
                nc.sync.dma_start(xT[:], d_x)
                w0 = stat.tile([IN + 1, G4], fp16)
                nc.sync.dma_start(w0[:], d_w0)
                Wx = stat.tile([128, 9 * G4], fp16)
                nc.sync.dma_start(Wx[:], d_wx)
                Wh = stat.tile([128, 10 * G4], fp16)
                nc.sync.dma_start(Wh[:], d_wh)
                bias4 = stat.tile([4, 3 * G4], fp16)
                nc.sync.dma_start(bias4[:], d_bias4)
                bsel = stat.tile([4, 128], fp16)
                nc.sync.dma_start(bsel[:], d_bsel)

                psg = [ctx.enter_context(
                    tc.tile_pool(name=f"ps{g}", bufs=1, space="PSUM"))
                    for g in range(3)]
                pst = ctx.enter_context(tc.tile_pool(name="pst", bufs=2,
                                                     space="PSUM"))
                actp = [ctx.enter_context(tc.tile_pool(name=f"act{g}", bufs=2))
                        for g in range(3)]
                hbp = [ctx.enter_context(tc.tile_pool(name=f"hb{g}", bufs=3))
                       for g in range(3)]
                htp = ctx.enter_context(tc.tile_pool(name="ht", bufs=2))
                mp = [ctx.enter_context(tc.tile_pool(name=f"m{g}", bufs=2))
                      for g in range(3)]
                tmpp = [ctx.enter_context(tc.tile_pool(name=f"tmp{g}", bufs=2))
                        for g in range(3)]
                thp = [ctx.enter_context(tc.tile_pool(name=f"th{g}", bufs=2))
                       for g in range(3)]
                cpp = ctx.enter_context(tc.tile_pool(name="cp", bufs=1))

                c_t = cpp.tile([128, 384], f32)
                prev_hb = [None, None, None]   # h_b tiles from previous tick
                hT = None

                for k in range(NT + 1):
                    # ---- bias preload into PSUM (off the critical chain) ----
                    act_l = [l for l in range(L) if 0 <= k - l < S_run]
                    gact = sorted(set(l // 4 for l in act_l))
                    ps = {}
                    for g in gact:
                        ps_g = psg[g].tile([128, G4], f32)
                        ps[g] = ps_g
                        nc.tensor.matmul(ps_g[:], bsel[:],
                                         bias4[:, G4 * g:G4 * (g + 1)],
                                         start=True, stop=False,
                                         skip_group_check=True)
                    # ---- phase A: transpose prev tick's h into hT ----
                    act_prev = [l for l in range(L) if 0 <= k - 1 - l < S_run]
                    gprev = sorted(set(l // 4 for l in act_prev))
                    if gprev:
                        hT = htp.tile([128, 384], fp16)
                        pt = pst.tile([128, 384], fp16)
                        for g in gprev:
                            nc.tensor.transpose(
                                pt[:, 128 * g:128 * (g + 1)],
                                prev_hb[g][:], ident[:])
                            nc.vector.tensor_copy(
                                hT[:, 128 * g:128 * (g + 1)],
                                pt[:, 128 * g:128 * (g + 1)])
                        if 9 in act_prev:
                            t9 = k - 1 - 9
                            nc.gpsimd.tensor_copy(
                                hs9[:, 32 * t9:32 * t9 + 32], hT[:, 288:320])
                    if k == NT:
                        break

                    # ---- phase B: matmuls per group ----
                    for g in gact:
                        ps_g = ps[g]
                        for l in act_l:
                            if l // 4 != g:
                                continue
                            t = k - l
                            m = l % 4
                            pr = slice(32 * m, 32 * m + 32)
                            o = ps_g[pr, :]
                            tp = (0, 32 * m)
                            if l == 0:
                                lx, wx_r = xT[:, 32 * t:32 * t + 32], w0[:]
                            else:
                                lx = hT[:, 32 * (l - 1):32 * (l - 1) + 32]
                                wx_r = Wx[:, (l - 1) * G4:l * G4]
                            nc.tensor.matmul(o, lx, wx_r, start=False,
                                             stop=(t == 0),
                                             tile_position=tp,
                                             skip_group_check=True)
                            if t > 0:
                                nc.tensor.matmul(
                                    o, hT[:, 32 * l:32 * l + 32],
                                    Wh[:, l * G4:(l + 1) * G4],
                                    start=False, stop=True, tile_position=tp,
                                    skip_group_check=True)

                    # ---- cells: pass 1 (gates + c update) ----
                    cell = {}
                    for g in gact:
                        full_g = all(
                            (4 * g + m in act_l and k - (4 * g + m) > 0)
                            for m in range(4) if 4 * g + m < L)
                        a = actp[g].tile([128, G4], f32)
                        mm_ = mp[g].tile([128, 128], f32)
                        tt = tmpp[g].tile([128, 128], f32)
                        th = thp[g].tile([128, 128], fp16)
                        h_b = hbp[g].tile([128, 128], fp16)
                        cc = c_t[:, 128 * g:128 * (g + 1)]
                        cell[g] = (a, mm_, tt, th, h_b, cc, full_g)
                        ps_g = ps[g]
                        if full_g:
                            # i,f,o = sigmoid; col 384:512 = sigmoid(2*zg)
                            # (g-weights pre-scaled 2x on host)
                            nc.scalar.activation(a[:], ps_g[:], Sig)
                            nc.vector.tensor_scalar(
                                a[:, 384:512], a[:, 384:512], 2.0, -1.0,
                                MUL, ADD)
                            nc.gpsimd.tensor_mul(tt[:], a[:, 0:128],
                                                 a[:, 384:512])
                            nc.vector.tensor_mul(cc, a[:, 128:256], cc)
                            nc.vector.tensor_add(cc, cc, tt[:])
                        else:
                            for l in act_l:
                                if l // 4 != g:
                                    continue
                                t = k - l
                                m = l % 4
                                pr = slice(32 * m, 32 * m + 32)
                                nc.scalar.activation(a[pr, :], ps_g[pr, :],
                                                     Sig)
                                nc.vector.tensor_scalar(
                                    a[pr, 384:512], a[pr, 384:512], 2.0, -1.0,
                                    MUL, ADD)
                                nc.gpsimd.tensor_mul(tt[pr, :], a[pr, 0:128],
                                                     a[pr, 384:512])
                                if t == 0:
                                    nc.vector.tensor_copy(cc[pr], tt[pr, :])
                                else:
                                    nc.vector.tensor_mul(cc[pr],
                                                         a[pr, 128:256],
                                                         cc[pr])
                                    nc.vector.tensor_add(cc[pr], cc[pr],
                                                         tt[pr, :])
                    # ---- cells: pass 2 (tanh(c), h = o*tanh(c)) ----
                    for g in gact:
                        a, mm_, tt, th, h_b, cc, full_g = cell[g]
                        if full_g:
                            nc.scalar.activation(th[:], cc, Tanh)
                            nc.gpsimd.tensor_mul(h_b[:], a[:, 256:384], th[:])
                        else:
                            for l in act_l:
                                if l // 4 != g:
                                    continue
                                m = l % 4
                                pr = slice(32 * m, 32 * m + 32)
                                nc.scalar.activation(th[pr, :], cc[pr], Tanh)
                                nc.gpsimd.tensor_mul(h_b[pr, :],
                                                     a[pr, 256:384],
                                                     th[pr, :])
                        prev_hb[g] = h_b

            if d_hs9 is not None:
                nc.sync.dma_start(d_hs9, hs9[:])
            # ================= attention + FC =================
            with ExitStack() as ctx:
                st2 = ctx.enter_context(tc.tile_pool(name="st2", bufs=1))
                ps2 = ctx.enter_context(tc.tile_pool(name="ps2", bufs=2,
                                                     space="PSUM"))
                sc2 = ctx.enter_context(tc.tile_pool(name="sc2", bufs=2))
                aw = st2.tile([128, 128], fp16)
                nc.sync.dma_start(aw[:], d_attn)
                ab = st2.tile([128, 1], f32)
                nc.sync.dma_start(ab[:], d_attnb)
                vw = st2.tile([128, 1], fp16)
                nc.sync.dma_start(vw[:], d_vw)
                fcw = st2.tile([128, OUT], f32)
                nc.sync.dma_start(fcw[:], d_fcw)
                fcb = st2.tile([1, OUT], f32)
                nc.sync.dma_start(fcb[:], d_fcb)
                ones128 = st2.tile([1, 128], fp16)
                nc.vector.memset(ones128[:], 1.0)
                ones128f = st2.tile([1, 128], f32)
                nc.vector.memset(ones128f[:], 1.0)
                onesBC = st2.tile([1, BC], f32)
                nc.vector.memset(onesBC[:], 1.0)

                NCH = (S_run * BC) // 512
                wgt = st2.tile([1, S_run * BC], fp16)
                for ch in range(NCH):
                    cs = slice(512 * ch, 512 * (ch + 1))
                    pa = ps2.tile([128, 512], f32, tag="big")
                    nc.tensor.matmul(pa[:], aw[:], hs9[:, cs],
                                     start=True, stop=True)
                    sc = sc2.tile([128, 512], fp16)
                    nc.scalar.activation(sc[:], pa[:], Tanh, bias=ab[:])
                    pl = ps2.tile([1, 512], f32, tag="pl")
                    nc.tensor.matmul(pl[:], vw[:], sc[:],
                                     start=True, stop=True)
                    nc.scalar.activation(wgt[:, cs], pl[:], Exp)
                # unnormalized weighted sum + per-b normalization at the end
                sm = st2.tile([1, BC], f32)
                nc.vector.tensor_reduce(
                    sm[:], wgt[:].rearrange("p (t b) -> p b t", b=BC),
                    axis=mybir.AxisListType.X, op=mybir.AluOpType.add)
                rsm = st2.tile([1, BC], f32)
                nc.vector.reciprocal(rsm[:], sm[:])

                parts = st2.tile([128, NCH * BC], f32)
                for ch in range(NCH):
                    cs = slice(512 * ch, 512 * (ch + 1))
                    pw = ps2.tile([128, 512], f32, tag="big")
                    nc.tensor.matmul(pw[:], ones128[:], wgt[:, cs],
                                     start=True, stop=True)
                    wp = sc2.tile([128, 512], f32, tag="wp")
                    nc.vector.tensor_mul(wp[:], hs9[:, cs], pw[:])
                    nc.vector.tensor_reduce(
                        parts[:, BC * ch:BC * (ch + 1)],
                        wp[:].rearrange("p (t b) -> p b t", b=BC),
                        axis=mybir.AxisListType.X, op=mybir.AluOpType.add)
                ctxv = st2.tile([128, BC], f32)
                nc.vector.tensor_reduce(
                    ctxv[:], parts[:].rearrange("p (c b) -> p b c", b=BC),
                    axis=mybir.AxisListType.X, op=mybir.AluOpType.add)
                prn = ps2.tile([128, BC], f32, tag="pl")
                nc.tensor.matmul(prn[:], ones128f[:], rsm[:],
                                 start=True, stop=True)
                nc.vector.tensor_mul(ctxv[:], ctxv[:], prn[:])

                pf = ps2.tile([OUT, BC], f32, tag="pl")
                nc.tensor.matmul(pf[:], fcw[:], ctxv[:],
                                 start=True, stop=False)
                nc.tensor.matmul(pf[:], fcb[:], onesBC[:],
                                 start=False, stop=True)
                ov = sc2.tile([OUT, BC], f32, tag="ov")
                nc.vector.tensor_copy(ov[:], pf[:])
                nc.sync.dma_start(d_out, ov[:])

    nc.compile()
    return nc


def _prep_inputs(x, w_ih0, w_ih, w_hh, b_ih, b_hh, attn_w, attn_b, v_w, v_b,
                 fc_w, fc_b, S_run):
    hf = np.float16
    perm = np.concatenate([np.arange(0, H), np.arange(H, 2 * H),
                           np.arange(3 * H, 4 * H), np.arange(2 * H, 3 * H)])
    # scale the g-gate weights/bias by 2: tanh(z) = 2*sigmoid(2z) - 1 is
    # evaluated through the sigmoid table
    gsc = np.ones((4 * H,), np.float32)
    gsc[384:512] = 2.0
    w0 = np.concatenate([w_ih0.T, np.zeros((1, 4 * H), np.float32)],
                        0)[:, perm] * gsc
    wx = np.concatenate([w_ih[l - 1].T[:, perm] * gsc for l in range(1, L)], 1)
    wh = np.concatenate([w_hh[l].T[:, perm] * gsc for l in range(L)], 1)
    bias4 = np.zeros((4, 3 * G4), np.float32)
    for l in range(L):
        g, m = l // 4, l % 4
        bias4[m, G4 * g:G4 * (g + 1)] = (b_ih[l] + b_hh[l])[perm] * gsc
    bsel = np.zeros((4, 128), np.float32)
    for m in range(4):
        bsel[m, 32 * m:32 * m + 32] = 1.0
    shared = {
        "w0": np.ascontiguousarray(w0).astype(hf),
        "wx": np.ascontiguousarray(wx).astype(hf),
        "wh": np.ascontiguousarray(wh).astype(hf),
        "bias4": np.ascontiguousarray(bias4).astype(hf),
        "bsel": np.ascontiguousarray(bsel).astype(hf),
        "attn_wT": np.ascontiguousarray(attn_w.T).astype(hf),
        "attn_b": np.ascontiguousarray(attn_b[:, None], np.float32),
        "v_w": np.ascontiguousarray(v_w.T).astype(hf),
        "fc_wT": np.ascontiguousarray(fc_w.T, np.float32),
        "fc_b": np.ascontiguousarray(fc_b[None, :], np.float32),
    }
    in_maps = []
    for c in range(NCORES):
        xs = x[c * BC:(c + 1) * BC, :S_run, :]
        xt = np.transpose(xs, (2, 1, 0)).reshape(IN, S_run * BC)
        xt = np.concatenate([xt, np.ones((1, S_run * BC), np.float32)], 0)
        m = dict(shared)
        m["x"] = np.ascontiguousarray(xt).astype(hf)
        in_maps.append(m)
    return in_maps


def run(inputs, S_run=S, trace=False):
    from concourse import bass_utils
    if S_run not in _CACHE:
        _CACHE[S_run] = _build(S_run)
    nc = _CACHE[S_run]
    in_maps = _prep_inputs(S_run=S_run, **inputs)
    res = bass_utils.run_bass_kernel_spmd(
        nc, in_maps, core_ids=list(range(NCORES)), trace=trace)
    out = np.concatenate([np.asarray(res.results[c]["out"], np.float32).T
                          for c in range(NCORES)], 0)
    return np.ascontiguousarray(out, np.float32), res


def kernel(**inputs):
    inputs = {k: np.asarray(v, np.float32) for k, v in inputs.items()}
    out, _ = run(inputs, S_run=S)
    return out
